# revision 1
# baseline (speedup 1.0000x reference)
"""Trainium2 kernel for nn_Loss4 (topk_masking).

reference:
    x_no_y = x.at[arange(B), y].set(0.0)
    s_topk = top_k(x_no_y, 5)           # [B, 5]
    s_y    = x[arange(B), y]            # [B]
    m      = mean(s_topk, -1)           # [B]
    out    = mean(relu(1 + m[None,:] - s_y[:,None]))   # scalar

Strategy: the only heavy part is the per-row top-k over [4096, 50257] f32
(823 MB streamed once).  Shard rows across 8 cores (512 rows each); on each
core stream the shard through the DVE MAX8 instruction (`nc.vector.max` =
top-8 per partition) hierarchically: top-8 per column-chunk, then top-8 of
the concatenated chunk results.  Device output is the exact per-row top-8
multiset of raw x ([4096, 8] total, 128 KB).

Host side (negligible):  s_y gather; top-5 of x_no_y is recovered exactly
from (top-8 of x, s_y): if s_y >= 8th largest, drop one instance equal to
s_y, then merge the value 0.0 (the scattered entry) and take the first 5.
The final [B,B] mean decomposes per row via sorting s_y + prefix sums:
sum_j relu(a_i - s_y_j) = cnt_i * a_i - prefixsum(s_y)[cnt_i],
cnt_i = #{j : s_y_j < a_i},  a_i = 1 + m_i.
"""

import numpy as np

B = 4096
C = 50257
K = 5
N_CORES = 8
R_PER_CORE = B // N_CORES       # 512 rows per core
P = 128                         # SBUF partitions
N_RG = R_PER_CORE // P          # 4 row-groups per core
# column chunks (DVE max8 free-size limit is 16384; ~25KB/partition tiles)
_CHUNKS = [6283] * 7 + [6276]
assert sum(_CHUNKS) == C and all(8 <= c <= 16384 for c in _CHUNKS)

_CACHE = {}


def _build_nc(repeat=1):
    import concourse.bacc as bacc
    import concourse.mybir as mybir
    import concourse.tile as tile

    nc = bacc.Bacc(None, enable_partition_id=False)
    f32 = mybir.dt.float32
    x = nc.declare_dram_parameter("x", [R_PER_CORE, C], f32, isOutput=False)
    out = nc.declare_dram_parameter("top8", [R_PER_CORE, 8], f32, isOutput=True)
    n_ch = len(_CHUNKS)
    with tile.TileContext(nc) as tc:
        with (
            tc.tile_pool(name="data", bufs=6) as dpool,
            tc.tile_pool(name="res", bufs=3) as rpool,
        ):
            for _rep in range(repeat):
                for rg in range(N_RG):
                    r0 = rg * P
                    stage1 = rpool.tile([P, 8 * n_ch], f32, tag="stage1")
                    final8 = rpool.tile([P, 8], f32, tag="final8")
                    c0 = 0
                    for ci, csz in enumerate(_CHUNKS):
                        t = dpool.tile([P, _CHUNKS[0]], f32, tag="chunk")
                        nc.sync.dma_start(out=t[:, :csz], in_=x[r0 : r0 + P, c0 : c0 + csz])
                        nc.vector.max(stage1[:, ci * 8 : (ci + 1) * 8], t[:, :csz])
                        c0 += csz
                    nc.vector.max(final8[:, :], stage1[:, :])
                    nc.sync.dma_start(out=out[r0 : r0 + P, :], in_=final8[:, :])
    nc.finalize()
    return nc


def _get_runner(repeat=1):
    """Build (once) a persistent jitted 8-core runner: f(x_full[4096,C]) -> top8[4096,8]."""
    if repeat in _CACHE:
        return _CACHE[repeat]

    import jax
    import jax.numpy as jnp
    from jax.experimental.shard_map import shard_map
    from jax.sharding import Mesh, PartitionSpec

    from concourse import bass2jax
    from concourse.bass2jax import _bass_exec_p, install_neuronx_cc_hook

    install_neuronx_cc_hook()
    nc = _build_nc(repeat)
    assert nc.partition_id_tensor is None

    out_shape = (R_PER_CORE, 8)

    def _body(xc, zc):
        outs = _bass_exec_p.bind(
            xc,
            zc,
            out_avals=(jax.core.ShapedArray(out_shape, np.float32),),
            in_names=("x", "top8"),
            out_names=("top8",),
            lowering_input_output_aliases=(),
            sim_require_finite=True,
            sim_require_nnan=True,
            nc=nc,
        )
        return tuple(outs)

    devices = jax.devices()[:N_CORES]
    mesh = Mesh(np.asarray(devices), ("core",))
    sharded = jax.jit(
        shard_map(
            _body,
            mesh=mesh,
            in_specs=(PartitionSpec("core"), PartitionSpec("core")),
            out_specs=(PartitionSpec("core"),),
            check_rep=False,
        ),
        donate_argnums=(1,),
        keep_unused=True,
    )

    def run(x_full):
        zeros = np.zeros((B, 8), np.float32)
        (o,) = sharded(x_full, zeros)
        return np.asarray(o)

    _CACHE[repeat] = (run, sharded, mesh)
    return _CACHE[repeat]


def _device_top8(x_full):
    run, _, _ = _get_runner(1)
    return run(x_full)


def _finalize(top8, x, y):
    """Exact host-side finish from per-row top-8 of raw x."""
    b = x.shape[0]
    s_y = x[np.arange(b), y]                      # [B] f32, bit-exact row gather
    t8 = np.sort(top8, axis=1)[:, ::-1]           # descending, [B, 8]
    in_top = s_y >= t8[:, 7]
    # drop ONE instance equal to s_y in rows where the y-entry is in the top-8
    eq = (t8 == s_y[:, None]) & in_top[:, None]
    first = eq & (np.cumsum(eq, axis=1) == 1)
    t8_mod = np.where(first, -np.inf, t8)
    # candidates for top-5 of x_no_y: remaining top-8 entries plus the
    # scattered 0.0 at the y position
    cand = np.concatenate([t8_mod, np.zeros((b, 1), np.float32)], axis=1)
    cand = np.sort(cand, axis=1)[:, ::-1]
    top5 = cand[:, :K].astype(np.float32)
    m = top5.mean(axis=1)                         # [B]

    a = 1.0 + m.astype(np.float64)                # [B]
    s = np.sort(s_y.astype(np.float64))
    ps = np.concatenate([[0.0], np.cumsum(s)])
    cnt = np.searchsorted(s, a, side="left")      # #{j: s_y_j < a_i}
    total = float((cnt * a - ps[cnt]).sum())
    return np.asarray(total / (b * b), dtype=np.float32)


def kernel(x, y):
    x = np.ascontiguousarray(np.asarray(x, dtype=np.float32))
    y = np.asarray(y).astype(np.int64)
    top8 = _device_top8(x)
    return _finalize(top8, x, y)



# revision 4
# speedup vs baseline: 92.2517x; 92.2517x over previous
"""Trainium2 kernel for nn_Loss4 (topk_masking).

reference:
    x_no_y = x.at[arange(B), y].set(0.0)
    s_topk = top_k(x_no_y, 5)           # [B, 5]
    s_y    = x[arange(B), y]            # [B]
    m      = mean(s_topk, -1)           # [B]
    out    = mean(relu(1 + m[None,:] - s_y[:,None]))   # scalar

Strategy: the only heavy part is the per-row top-k over [4096, 50257] f32
(823 MB streamed once).  Shard rows across 8 cores (512 rows each); on each
core stream the shard through the DVE MAX8 instruction (`nc.vector.max` =
top-8 per partition) hierarchically: top-8 per column-chunk, then top-8 of
the concatenated chunk results.  Device output is the exact per-row top-8
multiset of raw x ([4096, 8] total, 128 KB).

The kernel is HBM-bandwidth-bound (102.9 MB/core at the ~358 GB/s per-core
share of the 4 HBM stacks = ~287 us floor).  The input-chunk DMAs stream on
the sync-engine HWDGE queue; the tiny per-row-group top-8 writeback issues
from the scalar engine's separate HWDGE queue so its wait on the DVE result
never stalls the input stream between row groups.

Host side (negligible):  s_y gather; top-5 of x_no_y is recovered exactly
from (top-8 of x, s_y): if s_y >= 8th largest, drop one instance equal to
s_y, then merge the value 0.0 (the scattered entry) and take the first 5.
The final [B,B] mean decomposes per row via sorting s_y + prefix sums:
sum_j relu(a_i - s_y_j) = cnt_i * a_i - prefixsum(s_y)[cnt_i],
cnt_i = #{j : s_y_j < a_i},  a_i = 1 + m_i.
"""

import numpy as np

B = 4096
C = 50257
K = 5
N_CORES = 8
R_PER_CORE = B // N_CORES       # 512 rows per core
P = 128                         # SBUF partitions
N_RG = R_PER_CORE // P          # 4 row-groups per core
# column chunks (DVE max8 free-size limit is 16384; ~50KB/partition tiles --
# fewer, bigger DMAs measured fastest on HW: 289.7us vs 292.0us with 25KB)
_CHUNKS = [12565] * 3 + [12562]
assert sum(_CHUNKS) == C and all(8 <= c <= 16384 for c in _CHUNKS)
_DBUFS = 4  # 4 x 50.26KB = 201KB of the ~208KB usable per partition

_CACHE = {}


def _build_nc(repeat=1):
    import concourse.bacc as bacc
    import concourse.mybir as mybir
    import concourse.tile as tile

    nc = bacc.Bacc(None, enable_partition_id=False)
    f32 = mybir.dt.float32
    x = nc.declare_dram_parameter("x", [R_PER_CORE, C], f32, isOutput=False)
    out = nc.declare_dram_parameter("top8", [R_PER_CORE, 8], f32, isOutput=True)
    n_ch = len(_CHUNKS)
    with tile.TileContext(nc) as tc:
        with (
            tc.tile_pool(name="data", bufs=_DBUFS) as dpool,
            tc.tile_pool(name="res", bufs=3) as rpool,
        ):
            for _rep in range(repeat):
                for rg in range(N_RG):
                    r0 = rg * P
                    stage1 = rpool.tile([P, 8 * n_ch], f32, tag="stage1")
                    final8 = rpool.tile([P, 8], f32, tag="final8")
                    c0 = 0
                    for ci, csz in enumerate(_CHUNKS):
                        t = dpool.tile([P, _CHUNKS[0]], f32, tag="chunk")
                        nc.sync.dma_start(out=t[:, :csz], in_=x[r0 : r0 + P, c0 : c0 + csz])
                        nc.vector.max(stage1[:, ci * 8 : (ci + 1) * 8], t[:, :csz])
                        c0 += csz
                    nc.vector.max(final8[:, :], stage1[:, :])
                    # writeback on the scalar engine's HWDGE queue: its wait on
                    # the DVE top-8 must not stall the sync-engine input stream
                    nc.scalar.dma_start(out=out[r0 : r0 + P, :], in_=final8[:, :])
    nc.finalize()
    return nc


def _get_runner(repeat=1):
    """Build (once) a persistent jitted 8-core runner: f(x_full[4096,C]) -> top8[4096,8]."""
    if repeat in _CACHE:
        return _CACHE[repeat]

    import jax
    from jax.experimental.shard_map import shard_map
    from jax.sharding import Mesh, PartitionSpec

    from concourse.bass2jax import _bass_exec_p, install_neuronx_cc_hook

    install_neuronx_cc_hook()
    nc = _build_nc(repeat)
    assert nc.partition_id_tensor is None

    out_shape = (R_PER_CORE, 8)

    def _body(xc, zc):
        outs = _bass_exec_p.bind(
            xc,
            zc,
            out_avals=(jax.core.ShapedArray(out_shape, np.float32),),
            in_names=("x", "top8"),
            out_names=("top8",),
            lowering_input_output_aliases=(),
            sim_require_finite=True,
            sim_require_nnan=True,
            nc=nc,
        )
        return tuple(outs)

    devices = jax.devices()[:N_CORES]
    mesh = Mesh(np.asarray(devices), ("core",))
    sharded = jax.jit(
        shard_map(
            _body,
            mesh=mesh,
            in_specs=(PartitionSpec("core"), PartitionSpec("core")),
            out_specs=(PartitionSpec("core"),),
            check_rep=False,
        ),
        donate_argnums=(1,),
        keep_unused=True,
    )

    def run(x_full):
        zeros = np.zeros((B, 8), np.float32)
        (o,) = sharded(x_full, zeros)
        return np.asarray(o)

    _CACHE[repeat] = (run, sharded, mesh)
    return _CACHE[repeat]


def _device_top8(x_full):
    run, _, _ = _get_runner(1)
    return run(x_full)


def _finalize(top8, x, y):
    """Exact host-side finish from per-row top-8 of raw x."""
    b = x.shape[0]
    s_y = x[np.arange(b), y]                      # [B] f32, bit-exact row gather
    t8 = np.sort(top8, axis=1)[:, ::-1]           # descending, [B, 8]
    in_top = s_y >= t8[:, 7]
    # drop ONE instance equal to s_y in rows where the y-entry is in the top-8
    eq = (t8 == s_y[:, None]) & in_top[:, None]
    first = eq & (np.cumsum(eq, axis=1) == 1)
    t8_mod = np.where(first, -np.inf, t8)
    # candidates for top-5 of x_no_y: remaining top-8 entries plus the
    # scattered 0.0 at the y position
    cand = np.concatenate([t8_mod, np.zeros((b, 1), np.float32)], axis=1)
    cand = np.sort(cand, axis=1)[:, ::-1]
    top5 = cand[:, :K].astype(np.float32)
    m = top5.mean(axis=1)                         # [B]

    a = 1.0 + m.astype(np.float64)                # [B]
    s = np.sort(s_y.astype(np.float64))
    ps = np.concatenate([[0.0], np.cumsum(s)])
    cnt = np.searchsorted(s, a, side="left")      # #{j: s_y_j < a_i}
    total = float((cnt * a - ps[cnt]).sum())
    return np.asarray(total / (b * b), dtype=np.float32)


def kernel(x, y):
    x = np.ascontiguousarray(np.asarray(x, dtype=np.float32))
    y = np.asarray(y).astype(np.int64)
    top8 = _device_top8(x)
    return _finalize(top8, x, y)
